# revision 2
# baseline (speedup 1.0000x reference)
"""Trainium2 Bass kernel for EquidistantDiscreteContinuousConv3d.

Math: out = conv3d(x, einsum('ogk,kzyx->ogzyx', weight, psi_local), stride 2,
pad 2) + bias, with x [2,8,128,128,128] -> out [2,16,64,64,64].

The dense 5^3 kernel only has taps within Euclidean radius 2 (33 of 125
offsets are nonzero). Sharding: 8 cores = batch(2) x y-quarters(4); each core
computes out[b, :, :, 16gy:16gy+16] from a y-overlapping, zero-padded input
slab spanning the FULL z range. No collectives - halos materialize as
overlapping host-side slices.

Device mapping: the tensor engine contracts K = (z_local(16) x ic(8)) = 128
partitions, with M = (oz_sub(8, 6 used) x oc(16)) packed into a block-banded
weight matrix (band encodes the 5 dz taps), looped over the 13 (dy, dx)
stencil taps that accumulate in PSUM. Full-z-per-core means 11 z-windows of 6
output planes each (vs 3 windows over a z-quarter), which amortizes the
window padding: 22 groups x 13 matmuls of N=512 = 146,432 PE columns/core
instead of 159,744. rhs slices come from a phase-decomposed (even/odd y and
x) view of each 16-plane window tile; the banded wc from the z-quarter layout
carries over unchanged (window-local plane index 2*ozs+dzi is layout
invariant).

Input arrives as 11 window tiles [128, 36*132] (z-overlap of 4 planes between
consecutive tiles is re-fetched), each as two non-overlapping half-DMAs
(yo rows [0,11) and [11,18)) so the first matmul starts early and per-kind
semaphore counts stay completion-exact (same-kind halves alternate two sems).

Raw Bacc pipeline per core (static, fully unrolled; no TileContext):
  ACT : wtile, A0, B0, A1, B1, B2 DMAs (critical prefix, ACT exits the
        preamble ~2.5us before SP), then 22 output DMAs interleaved with B3-10
  SP  : A2..A10 paced input half-DMAs, then end-of-run sem clear
  PE  : 56 N=64 warmups (clock ramp), then 22 groups x 13 banded matmuls
        accumulating in psum bank g%8
  DVE : 22 psum->stage copies (4 rotating stage slots)
"""

import os

import ml_dtypes
import numpy as np

BF16 = ml_dtypes.bfloat16

IC, OC = 8, 16
TAPS_XY = [
    (dy, dx) for dy in range(-2, 3) for dx in range(-2, 3) if dy * dy + dx * dx <= 4
]  # 13 taps
NW = 11  # z-windows of 6 (last: 4) output planes
NG = 2 * NW  # groups: g = 2*w + t, t = y-half of the 16-row output quarter
SUB_FREE = 36 * 132  # window tile free size: (yo 18, yp 2, px 2, xe 66)
ROW = 2 * 2 * 66  # one yo row = (yp, px, xe) block of 264 elements
A_ROWS = 11  # half A = yo [0,11): everything group t=0 touches
NSLOT = 6
N_CORES = 8

_MODULE = None
LAST_RESULT = None  # BassKernelResults of the most recent run (for test harness)


def _oz_per(w):
    return 6 if w < NW - 1 else 4


def _build_module():
    from contextlib import ExitStack

    import concourse.bacc as bacc
    import concourse.mybir as mybir

    f32 = mybir.dt.float32
    bf16 = mybir.dt.bfloat16

    nc = bacc.Bacc()
    x_in = nc.dram_tensor("xc", [NW, 128, SUB_FREE], bf16, kind="ExternalInput")
    w_in = nc.dram_tensor("wc", [128, 13 * 128], bf16, kind="ExternalInput")
    out = nc.dram_tensor("out", [64, 16, 16, 64], f32, kind="ExternalOutput")

    with ExitStack() as ctx:
        wsem = ctx.enter_context(nc.semaphore("wsem"))
        xsA = [ctx.enter_context(nc.semaphore(f"xsemA{i}")) for i in range(2)]
        xsB = [ctx.enter_context(nc.semaphore(f"xsemB{i}")) for i in range(2)]
        pesem = ctx.enter_context(nc.semaphore("pesem"))
        dvsem = ctx.enter_context(nc.semaphore("dvsem"))
        osem = ctx.enter_context(nc.semaphore("osem"))
        wtile = ctx.enter_context(nc.sbuf_tensor("wtile", [128, 13 * 128], bf16))
        xts = [
            ctx.enter_context(nc.sbuf_tensor(f"xt{i}", [128, SUB_FREE], bf16))
            for i in range(NSLOT)
        ]
        stgs = [
            ctx.enter_context(nc.sbuf_tensor(f"stg{i}", [128, 512], f32))
            for i in range(4)
        ]
        pss = [
            ctx.enter_context(nc.psum_tensor(f"ps{i}", [128, 512], f32))
            for i in range(8)
        ]
        x5s = [
            t[:].rearrange("p (a b d c) -> p a b d c", a=18, b=2, d=2, c=66)
            for t in xts
        ]

        def adma(eng, i):
            # pace: same-sem predecessor (i-2) must be complete so counts stay
            # completion-exact; slot i%NSLOT must be drained by the PE
            if i >= 2:
                eng.wait_ge(xsA[i % 2], 16 * (i // 2))
            if i >= NSLOT:
                eng.wait_ge(pesem, 2 * (i - NSLOT) + 2)
            eng.dma_start(
                out=xts[i % NSLOT][:, 0 : A_ROWS * ROW],
                in_=x_in[i, :, 0 : A_ROWS * ROW],
            ).then_inc(xsA[i % 2], 16)

        def bdma(eng, i):
            if i >= 2:
                eng.wait_ge(xsB[i % 2], 16 * (i // 2))
            if i >= NSLOT:
                eng.wait_ge(pesem, 2 * (i - NSLOT) + 2)
            eng.dma_start(
                out=xts[i % NSLOT][:, A_ROWS * ROW : SUB_FREE],
                in_=x_in[i, :, A_ROWS * ROW : SUB_FREE],
            ).then_inc(xsB[i % 2], 16)

        with nc.Block() as block:

            @block.scalar
            def _(act):
                act.dma_start(out=wtile[:], in_=w_in[:]).then_inc(wsem, 16)
                adma(act, 0)
                bdma(act, 0)
                adma(act, 1)
                bdma(act, 1)
                bdma(act, 2)

                def odma(s):
                    w, t = divmod(s, 2)
                    M = _oz_per(w) * 16
                    act.wait_ge(dvsem, s + 1)
                    dst = out[
                        6 * w : 6 * w + _oz_per(w), :, 8 * t : 8 * t + 8, :
                    ].rearrange("a b c d -> (a b) (c d)")
                    act.dma_start(out=dst, in_=stgs[s % 4][:M, :]).then_inc(osem, 16)

                # interleave B-half inputs with outputs so neither starves;
                # preceding odma's dvsem wait implies B(k)'s slot is free
                k = 3
                for s in range(NG):
                    odma(s)
                    if s % 2 == 1 and k < NW:
                        bdma(act, k)
                        k += 1

            @block.sync
            def _(sp):
                for i in range(2, NW):
                    adma(sp, i)
                # re-execution safety: clear sems once everything is done
                sp.wait_ge(osem, 16 * NG)
                for sem in (wsem, xsA[0], xsA[1], xsB[0], xsB[1], pesem, dvsem, osem):
                    sp.sem_clear(sem)

            @block.tensor
            def _(pe):
                # warm-up: cheap N=64 throwaway matmuls keep PE busy from the
                # preamble until the first input lands, so the clock gate is
                # ramped for every real matmul. Inputs may be mid-DMA garbage;
                # psum bank 7 is discarded by its first start=True.
                for _ in range(56):
                    pe.matmul(
                        pss[7][:, 0:64], wtile[:, 0:128], wtile[:, 0:64],
                        start=True, stop=True,
                    )
                pe.wait_ge(wsem, 16)
                for g in range(NG):
                    w, t = divmod(g, 2)
                    pe.wait_ge(xsA[w % 2], 16 * (w // 2 + 1))
                    if t == 1:
                        pe.wait_ge(xsB[w % 2], 16 * (w // 2 + 1))
                    if g >= 8:
                        pe.wait_ge(dvsem, g - 7)  # psum bank g%8 evacuated
                    x5 = x5s[w % NSLOT]
                    ps = pss[g % 8]
                    for j, (dy, dx) in enumerate(TAPS_XY):
                        jy, py = divmod(dy + 2, 2)
                        jx, px = divmod(dx + 2, 2)
                        a0 = 8 * t + jy
                        rhs = x5[
                            :, a0 : a0 + 8, py : py + 1, px : px + 1, jx : jx + 64
                        ]
                        mm = pe.matmul(
                            ps[:],
                            wtile[:, j * 128 : (j + 1) * 128],
                            rhs,
                            start=(j == 0),
                            stop=(j == len(TAPS_XY) - 1),
                        )
                        if j == len(TAPS_XY) - 1:
                            mm.then_inc(pesem, 1)

            @block.vector
            def _(dve):
                for g in range(NG):
                    M = _oz_per(g // 2) * 16
                    if g >= 4:
                        dve.wait_ge(osem, 16 * (g - 3))  # stage slot g%4 free
                    dve.wait_ge(pesem, g + 1)
                    dve.tensor_copy(
                        out=stgs[g % 4][:M, :], in_=pss[g % 8][:M]
                    ).then_inc(dvsem, 1)

    nc.compile()
    return nc


def _get_module():
    global _MODULE
    if _MODULE is None:
        _MODULE = _build_module()
    return _MODULE


def _band_weights(w5):
    """wc[k=(zl*8+ic), j*128 + ozs*16 + oc] block-banded weights (ozs 6,7 pad).

    Window-local: output plane ozs (0..5) of any window reads tile-local
    planes zl = 2*ozs + dzi; rows 15 and M-columns 96..127 stay zero."""
    wc = np.zeros((128, 13, 8, 16), np.float32)
    for j, (dy, dx) in enumerate(TAPS_XY):
        for dzi in range(5):
            dz = dzi - 2
            if dz * dz + dy * dy + dx * dx > 4:
                continue
            blk = w5[:, :, dzi, dy + 2, dx + 2].T  # [ic, oc]
            for ozs in range(6):
                zl = 2 * ozs + dzi
                wc[zl * 8 : (zl + 1) * 8, j, ozs, :] = blk
    return np.ascontiguousarray(wc.reshape(128, 13 * 128))


def _shard_core_input(x, b, gy):
    """Per-core padded input as 11 z-window tiles [128, 36*132]."""
    xp = np.zeros((IC, 136, 36, 132), BF16)
    y_lo = 32 * gy - 2
    src_lo, src_hi = max(0, y_lo), min(128, y_lo + 36)
    xp[:, 2:130, src_lo - y_lo : src_hi - y_lo, 2:130] = x[
        b, :, :, src_lo:src_hi, :
    ]
    tiles = np.empty((NW, 128, SUB_FREE), BF16)
    for w in range(NW):
        u = xp[:, 12 * w : 12 * w + 16]  # [ic, zl 16, y 36, x 132]
        # de-interleave phases: free = (yo 18, yp 2, px 2, xe 66)
        u = u.reshape(IC, 16, 36, 66, 2).transpose(0, 1, 2, 4, 3)
        u = u.reshape(IC, 16, 18, 2, 2, 66)
        tiles[w] = u.transpose(1, 0, 2, 3, 4, 5).reshape(128, SUB_FREE)
    return tiles


def kernel(x, weight, bias, psi_local):
    global LAST_RESULT
    from concourse.bass_utils import run_bass_kernel_spmd

    x = np.asarray(x, np.float32)
    weight = np.asarray(weight, np.float32)
    bias = np.asarray(bias, np.float32)
    psi_local = np.asarray(psi_local, np.float32)

    w5 = np.einsum("ogk,kzyx->ogzyx", weight, psi_local).astype(np.float32)
    wc = _band_weights(w5).astype(BF16)

    in_maps = []
    for core in range(N_CORES):
        b, gy = divmod(core, 4)
        in_maps.append({"xc": _shard_core_input(x, b, gy), "wc": wc})

    nc = _get_module()
    trace = bool(int(os.environ.get("KERNEL_TRACE", "0")))
    res = run_bass_kernel_spmd(
        nc, in_maps, core_ids=list(range(N_CORES)), trace=trace
    )
    LAST_RESULT = res

    out = np.empty((2, OC, 64, 64, 64), np.float32)
    for core in range(N_CORES):
        b, gy = divmod(core, 4)
        out[b, :, :, 16 * gy : 16 * gy + 16] = res.results[core]["out"].transpose(
            1, 0, 2, 3
        )
    out += bias[None, :, None, None, None]
    return out


# revision 4
# speedup vs baseline: 1.0140x; 1.0140x over previous
"""Trainium2 Bass kernel for EquidistantDiscreteContinuousConv3d.

Math: out = conv3d(x, einsum('ogk,kzyx->ogzyx', weight, psi_local), stride 2,
pad 2) + bias, with x [2,8,128,128,128] -> out [2,16,64,64,64].

The dense 5^3 kernel only has taps within Euclidean radius 2 (33 of 125
offsets are nonzero). Sharding: 8 cores = batch(2) x y-quarters(4); each core
computes out[b, :, :, 16gy:16gy+16] from a y-overlapping, zero-padded input
slab spanning the FULL z range. No collectives - halos materialize as
overlapping host-side slices.

Device mapping: the tensor engine contracts K = (z_local(16) x ic(8)) = 128
partitions, with M = (oz_sub(8, 6 used) x oc(16)) packed into a block-banded
weight matrix (band encodes the 5 dz taps), looped over the 13 (dy, dx)
stencil taps that accumulate in PSUM. Full-z-per-core means 11 z-windows of 6
output planes each (vs 3 windows over a z-quarter), which amortizes the
window padding: 22 groups x 13 matmuls of N=512 = 146,432 PE columns/core
instead of 159,744. rhs slices come from a phase-decomposed (even/odd y and
x) view of each 16-plane window tile; the banded wc from the z-quarter layout
carries over unchanged (window-local plane index 2*ozs+dzi is layout
invariant).

Input arrives as 11 window tiles [128, 36*132] (z-overlap of 4 planes between
consecutive tiles is re-fetched), each as two non-overlapping half-DMAs
(yo rows [0,11) and [11,18)) so the first matmul starts early and per-kind
semaphore counts stay completion-exact (same-kind halves alternate two sems).

Raw Bacc pipeline per core (static, fully unrolled; no TileContext):
  ACT : wtile, A0, B0, A1, B1, B2 DMAs (critical prefix, ACT exits the
        preamble ~2.5us before SP), then 22 output DMAs interleaved with B3-10
  SP  : A2..A10 paced input half-DMAs, then end-of-run sem clear
  PE  : 56 N=64 warmups (clock ramp), then 22 groups x 13 banded matmuls
        accumulating in psum bank g%8
  DVE : 22 psum->stage copies (4 rotating stage slots)
"""

import os

import ml_dtypes
import numpy as np

BF16 = ml_dtypes.bfloat16

IC, OC = 8, 16
TAPS_XY = [
    (dy, dx) for dy in range(-2, 3) for dx in range(-2, 3) if dy * dy + dx * dx <= 4
]  # 13 taps
NW = 11  # z-windows of 6 (last: 4) output planes
NG = 2 * NW  # groups: g = 2*w + t, t = y-half of the 16-row output quarter
SUB_FREE = 36 * 132  # window tile free size: (yo 18, yp 2, px 2, xe 66)
ROW = 2 * 2 * 66  # one yo row = (yp, px, xe) block of 264 elements
A_ROWS = 11  # half A = yo [0,11): everything group t=0 touches
NSLOT = 6
N_CORES = 8

_MODULE = None
LAST_RESULT = None  # BassKernelResults of the most recent run (for test harness)


def _oz_per(w):
    return 6 if w < NW - 1 else 4


def _build_module():
    from contextlib import ExitStack

    import concourse.bacc as bacc
    import concourse.mybir as mybir

    f32 = mybir.dt.float32
    bf16 = mybir.dt.bfloat16

    nc = bacc.Bacc()
    x_in = nc.dram_tensor("xc", [NW, 128, SUB_FREE], bf16, kind="ExternalInput")
    w_in = nc.dram_tensor("wc", [128, 13 * 128], bf16, kind="ExternalInput")
    out = nc.dram_tensor("out", [64, 16, 16, 64], f32, kind="ExternalOutput")

    with ExitStack() as ctx:
        wsem = ctx.enter_context(nc.semaphore("wsem"))
        xsA = [ctx.enter_context(nc.semaphore(f"xsemA{i}")) for i in range(2)]
        xsB = [ctx.enter_context(nc.semaphore(f"xsemB{i}")) for i in range(2)]
        pesem = ctx.enter_context(nc.semaphore("pesem"))
        dvsem = ctx.enter_context(nc.semaphore("dvsem"))
        osem = ctx.enter_context(nc.semaphore("osem"))
        wtile = ctx.enter_context(nc.sbuf_tensor("wtile", [128, 13 * 128], bf16))
        xts = [
            ctx.enter_context(nc.sbuf_tensor(f"xt{i}", [128, SUB_FREE], bf16))
            for i in range(NSLOT)
        ]
        stgs = [
            ctx.enter_context(nc.sbuf_tensor(f"stg{i}", [128, 512], f32))
            for i in range(4)
        ]
        pss = [
            ctx.enter_context(nc.psum_tensor(f"ps{i}", [128, 512], f32))
            for i in range(8)
        ]
        x5s = [
            t[:].rearrange("p (a b d c) -> p a b d c", a=18, b=2, d=2, c=66)
            for t in xts
        ]

        def adma(eng, i):
            # pace: same-sem predecessor (i-2) must be complete so counts stay
            # completion-exact; slot i%NSLOT must be drained by the PE
            if i >= 2:
                eng.wait_ge(xsA[i % 2], 16 * (i // 2))
            if i >= NSLOT:
                eng.wait_ge(pesem, 2 * (i - NSLOT) + 2)
            eng.dma_start(
                out=xts[i % NSLOT][:, 0 : A_ROWS * ROW],
                in_=x_in[i, :, 0 : A_ROWS * ROW],
            ).then_inc(xsA[i % 2], 16)

        def bdma(eng, i):
            if i >= 2:
                eng.wait_ge(xsB[i % 2], 16 * (i // 2))
            if i >= NSLOT:
                eng.wait_ge(pesem, 2 * (i - NSLOT) + 2)
            eng.dma_start(
                out=xts[i % NSLOT][:, A_ROWS * ROW : SUB_FREE],
                in_=x_in[i, :, A_ROWS * ROW : SUB_FREE],
            ).then_inc(xsB[i % 2], 16)

        with nc.Block() as block:

            @block.scalar
            def _(act):
                act.dma_start(out=wtile[:], in_=w_in[:]).then_inc(wsem, 16)
                adma(act, 0)
                bdma(act, 0)
                # throttle: let tile 0 land at full bandwidth before queueing
                # more behind its completion increments
                act.wait_ge(xsA[0], 16)
                adma(act, 1)
                bdma(act, 1)
                bdma(act, 2)

                def odma(s):
                    w, t = divmod(s, 2)
                    M = _oz_per(w) * 16
                    act.wait_ge(dvsem, s + 1)
                    dst = out[
                        6 * w : 6 * w + _oz_per(w), :, 8 * t : 8 * t + 8, :
                    ].rearrange("a b c d -> (a b) (c d)")
                    act.dma_start(out=dst, in_=stgs[s % 4][:M, :]).then_inc(osem, 16)

                # interleave B-half inputs with outputs so neither starves;
                # preceding odma's dvsem wait implies B(k)'s slot is free
                k = 3
                for s in range(NG):
                    odma(s)
                    if s % 2 == 1 and k < NW:
                        bdma(act, k)
                        k += 1

            @block.sync
            def _(sp):
                for i in range(2, NW):
                    adma(sp, i)
                # re-execution safety: clear sems once everything is done
                sp.wait_ge(osem, 16 * NG)
                for sem in (wsem, xsA[0], xsA[1], xsB[0], xsB[1], pesem, dvsem, osem):
                    sp.sem_clear(sem)

            @block.tensor
            def _(pe):
                # warm-up: cheap N=64 throwaway matmuls keep PE busy from the
                # preamble until the first input lands, so the clock gate is
                # ramped for every real matmul. Inputs may be mid-DMA garbage;
                # psum bank 7 is discarded by its first start=True.
                for _ in range(104):
                    pe.matmul(
                        pss[7][:, 0:64], wtile[:, 0:128], wtile[:, 0:64],
                        start=True, stop=True,
                    )
                pe.wait_ge(wsem, 16)
                for g in range(NG):
                    w, t = divmod(g, 2)
                    pe.wait_ge(xsA[w % 2], 16 * (w // 2 + 1))
                    if t == 1:
                        pe.wait_ge(xsB[w % 2], 16 * (w // 2 + 1))
                    if g >= 8:
                        pe.wait_ge(dvsem, g - 7)  # psum bank g%8 evacuated
                    x5 = x5s[w % NSLOT]
                    ps = pss[g % 8]
                    for j, (dy, dx) in enumerate(TAPS_XY):
                        jy, py = divmod(dy + 2, 2)
                        jx, px = divmod(dx + 2, 2)
                        a0 = 8 * t + jy
                        rhs = x5[
                            :, a0 : a0 + 8, py : py + 1, px : px + 1, jx : jx + 64
                        ]
                        mm = pe.matmul(
                            ps[:],
                            wtile[:, j * 128 : (j + 1) * 128],
                            rhs,
                            start=(j == 0),
                            stop=(j == len(TAPS_XY) - 1),
                        )
                        if j == len(TAPS_XY) - 1:
                            mm.then_inc(pesem, 1)

            @block.vector
            def _(dve):
                for g in range(NG):
                    M = _oz_per(g // 2) * 16
                    if g >= 4:
                        dve.wait_ge(osem, 16 * (g - 3))  # stage slot g%4 free
                    dve.wait_ge(pesem, g + 1)
                    dve.tensor_copy(
                        out=stgs[g % 4][:M, :], in_=pss[g % 8][:M]
                    ).then_inc(dvsem, 1)

    nc.compile()
    return nc


def _get_module():
    global _MODULE
    if _MODULE is None:
        _MODULE = _build_module()
    return _MODULE


def _band_weights(w5):
    """wc[k=(zl*8+ic), j*128 + ozs*16 + oc] block-banded weights (ozs 6,7 pad).

    Window-local: output plane ozs (0..5) of any window reads tile-local
    planes zl = 2*ozs + dzi; rows 15 and M-columns 96..127 stay zero."""
    wc = np.zeros((128, 13, 8, 16), np.float32)
    for j, (dy, dx) in enumerate(TAPS_XY):
        for dzi in range(5):
            dz = dzi - 2
            if dz * dz + dy * dy + dx * dx > 4:
                continue
            blk = w5[:, :, dzi, dy + 2, dx + 2].T  # [ic, oc]
            for ozs in range(6):
                zl = 2 * ozs + dzi
                wc[zl * 8 : (zl + 1) * 8, j, ozs, :] = blk
    return np.ascontiguousarray(wc.reshape(128, 13 * 128))


def _shard_core_input(x, b, gy):
    """Per-core padded input as 11 z-window tiles [128, 36*132]."""
    xp = np.zeros((IC, 136, 36, 132), BF16)
    y_lo = 32 * gy - 2
    src_lo, src_hi = max(0, y_lo), min(128, y_lo + 36)
    xp[:, 2:130, src_lo - y_lo : src_hi - y_lo, 2:130] = x[
        b, :, :, src_lo:src_hi, :
    ]
    tiles = np.empty((NW, 128, SUB_FREE), BF16)
    for w in range(NW):
        u = xp[:, 12 * w : 12 * w + 16]  # [ic, zl 16, y 36, x 132]
        # de-interleave phases: free = (yo 18, yp 2, px 2, xe 66)
        u = u.reshape(IC, 16, 36, 66, 2).transpose(0, 1, 2, 4, 3)
        u = u.reshape(IC, 16, 18, 2, 2, 66)
        tiles[w] = u.transpose(1, 0, 2, 3, 4, 5).reshape(128, SUB_FREE)
    return tiles


def kernel(x, weight, bias, psi_local):
    global LAST_RESULT
    from concourse.bass_utils import run_bass_kernel_spmd

    x = np.asarray(x, np.float32)
    weight = np.asarray(weight, np.float32)
    bias = np.asarray(bias, np.float32)
    psi_local = np.asarray(psi_local, np.float32)

    w5 = np.einsum("ogk,kzyx->ogzyx", weight, psi_local).astype(np.float32)
    wc = _band_weights(w5).astype(BF16)

    in_maps = []
    for core in range(N_CORES):
        b, gy = divmod(core, 4)
        in_maps.append({"xc": _shard_core_input(x, b, gy), "wc": wc})

    nc = _get_module()
    trace = bool(int(os.environ.get("KERNEL_TRACE", "0")))
    res = run_bass_kernel_spmd(
        nc, in_maps, core_ids=list(range(N_CORES)), trace=trace
    )
    LAST_RESULT = res

    out = np.empty((2, OC, 64, 64, 64), np.float32)
    for core in range(N_CORES):
        b, gy = divmod(core, 4)
        out[b, :, :, 16 * gy : 16 * gy + 16] = res.results[core]["out"].transpose(
            1, 0, 2, 3
        )
    out += bias[None, :, None, None, None]
    return out


# revision 13
# speedup vs baseline: 1.3497x; 1.3310x over previous
"""Trainium2 Bass kernel for EquidistantDiscreteContinuousConv3d.

Math: out = conv3d(x, einsum('ogk,kzyx->ogzyx', weight, psi_local), stride 2,
pad 2) + bias, with x [2,8,128,128,128] -> out [2,16,64,64,64].

KEY STRUCTURE: although the basis nominally spans a 5^3 stencil, the
reference computes r = sqrt(d^2 + 1e-12), which pushes the six radius-2
offsets (+-2,0,0),(0,+-2,0),(0,0,+-2) infinitesimally OUTSIDE r_cutoff, so
psi (and hence the contracted kernel for ANY weights) is identically zero
there. The effective stencil is exactly the 3x3x3 cube (27 taps). This
kernel exploits that: 9 (dy,dx) passes with a 3-tap z-band instead of 13
passes with a 5-tap band.

Sharding: 8 cores = batch(2) x y-quarters(4); each core computes
out[b, :, :, 16gy:16gy+16] from a y-overlapping, zero-padded input slab
spanning the FULL z range. No collectives.

Device mapping: the tensor engine contracts K = (z_local(16) x ic(8)) = 128
partitions, with M = (oz_sub(8, 7 used) x oc(16)) packed into a block-banded
weight matrix (band encodes the 3 dz taps), looped over the 9 (dy, dx) taps
accumulating in PSUM. A 15-plane window supports 7 output planes -> 10
z-windows x 2 y-halves x 9 taps = 180 matmuls of N=512 per core. rhs slices
come from a phase-decomposed (even/odd y and x) view of each window tile.

Input arrives as 10 window tiles (15 z-planes = partitions 0..119; partition
rows 120-127 are zeroed by the first 6 transfers and never rewritten), each
as two non-overlapping half-DMAs (yo rows [0,9) and [9,17)). Output leaves
as bf16 (upcast on host) to halve write traffic.

Raw Bacc pipeline per core (static, fully unrolled; no TileContext):
  ACT : wtile, A0, B0 DMAs, throttle, A1, B1, B2, then 20 output DMAs
        interleaved with B3-B9
  SP  : A2..A9 paced input half-DMAs, then end-of-run sem clear
  PE  : 120 N=64 warmups (clock ramp), then 20 groups x 9 banded matmuls
        accumulating in psum bank g%8
  DVE : 20 psum->stage bf16 copies (4 rotating stage slots)
"""

import os

import ml_dtypes
import numpy as np

BF16 = ml_dtypes.bfloat16

IC, OC = 8, 16
TAPS_XY = [(dy, dx) for dy in (-1, 0, 1) for dx in (-1, 0, 1)]  # 9 taps
NW = 10  # z-windows of 7 (last: 1) output planes
NG = 2 * NW  # groups: g = 2*w + t, t = y-half of the 16-row output quarter
SUB_FREE = 36 * 132  # window tile free size: (yo 18, yp 2, px 2, xe 66)
ROW = 2 * 2 * 66  # one yo row = (yp, px, xe) block of 264 elements
A_END = 9 * ROW  # half A = yo [0,9): everything group t=0 touches
B_END = 17 * ROW  # half B = yo [9,17); row 17 is never read
NSLOT = 6
N_CORES = 8

_MODULE = None
LAST_RESULT = None  # BassKernelResults of the most recent run (for test harness)


def _oz_per(w):
    return 7 if w < NW - 1 else 1


def _build_module():
    from contextlib import ExitStack

    import concourse.bacc as bacc
    import concourse.mybir as mybir

    f32 = mybir.dt.float32
    bf16 = mybir.dt.bfloat16

    nc = bacc.Bacc()
    x_in = nc.dram_tensor("xc", [NW, 128, SUB_FREE], bf16, kind="ExternalInput")
    w_in = nc.dram_tensor("wc", [128, 9 * 128], bf16, kind="ExternalInput")
    out = nc.dram_tensor("out", [64, 16, 16, 64], bf16, kind="ExternalOutput")

    with ExitStack() as ctx:
        wsem = ctx.enter_context(nc.semaphore("wsem"))
        xsA = [ctx.enter_context(nc.semaphore(f"xsemA{i}")) for i in range(2)]
        xsB = [ctx.enter_context(nc.semaphore(f"xsemB{i}")) for i in range(2)]
        pesem = ctx.enter_context(nc.semaphore("pesem"))
        dvsem = ctx.enter_context(nc.semaphore("dvsem"))
        osem = ctx.enter_context(nc.semaphore("osem"))
        wtile = ctx.enter_context(nc.sbuf_tensor("wtile", [128, 9 * 128], bf16))
        xts = [
            ctx.enter_context(nc.sbuf_tensor(f"xt{i}", [128, SUB_FREE], bf16))
            for i in range(NSLOT)
        ]
        stgs = [
            ctx.enter_context(nc.sbuf_tensor(f"stg{i}", [128, 512], bf16))
            for i in range(4)
        ]
        pss = [
            ctx.enter_context(nc.psum_tensor(f"ps{i}", [128, 512], f32))
            for i in range(8)
        ]
        x5s = [
            t[:].rearrange("p (a b d c) -> p a b d c", a=18, b=2, d=2, c=66)
            for t in xts
        ]

        def adma(eng, i):
            # first NSLOT transfers carry host zeros into partition rows
            # 120-127 (never rewritten - the banded weights are zero there, so
            # they must not be NaN garbage); later tiles skip those rows.
            # pace: same-sem predecessor (i-2) must be complete so counts stay
            # completion-exact; slot i%NSLOT must be drained by the PE
            P = 128 if i < NSLOT else 120
            if i >= 2:
                eng.wait_ge(xsA[i % 2], 16 * (i // 2))
            if i >= NSLOT:
                eng.wait_ge(pesem, 2 * (i - NSLOT) + 2)
            eng.dma_start(
                out=xts[i % NSLOT][:P, 0:A_END],
                in_=x_in[i, 0:P, 0:A_END],
            ).then_inc(xsA[i % 2], 16)

        def bdma(eng, i):
            P = 128 if i < NSLOT else 120
            if i >= 2:
                eng.wait_ge(xsB[i % 2], 16 * (i // 2))
            if i >= NSLOT:
                eng.wait_ge(pesem, 2 * (i - NSLOT) + 2)
            eng.dma_start(
                out=xts[i % NSLOT][:P, A_END:B_END],
                in_=x_in[i, 0:P, A_END:B_END],
            ).then_inc(xsB[i % 2], 16)

        with nc.Block() as block:

            @block.scalar
            def _(act):
                act.dma_start(out=wtile[:], in_=w_in[:]).then_inc(wsem, 16)
                adma(act, 0)
                bdma(act, 0)
                # throttle: let tile 0 land at full bandwidth before queueing
                # more behind its completion increments
                act.wait_ge(xsA[0], 16)
                adma(act, 1)
                bdma(act, 1)
                bdma(act, 2)

                def odma(s):
                    w, t = divmod(s, 2)
                    M = _oz_per(w) * 16
                    act.wait_ge(dvsem, s + 1)
                    dst = out[
                        7 * w : 7 * w + _oz_per(w), :, 8 * t : 8 * t + 8, :
                    ].rearrange("a b c d -> (a b) (c d)")
                    act.dma_start(out=dst, in_=stgs[s % 4][:M, :]).then_inc(osem, 16)

                # interleave B-half inputs with outputs so neither starves;
                # preceding odma's dvsem wait implies B(k)'s slot is free
                k = 3
                for s in range(NG):
                    odma(s)
                    if s % 2 == 1 and k < NW:
                        bdma(act, k)
                        k += 1

            @block.sync
            def _(sp):
                for i in range(2, NW):
                    adma(sp, i)
                # re-execution safety: clear sems once everything is done
                sp.wait_ge(osem, 16 * NG)
                for sem in (wsem, xsA[0], xsA[1], xsB[0], xsB[1], pesem, dvsem, osem):
                    sp.sem_clear(sem)

            @block.tensor
            def _(pe):
                # warm-up: cheap N=64 throwaway matmuls keep PE busy from the
                # preamble until the first input lands, so the clock gate is
                # ramped for every real matmul. Inputs may be mid-DMA garbage;
                # psum bank 7 is discarded by its first start=True.
                for _ in range(120):
                    pe.matmul(
                        pss[7][:, 0:64], wtile[:, 0:128], wtile[:, 0:64],
                        start=True, stop=True,
                    )
                pe.wait_ge(wsem, 16)
                for g in range(NG):
                    w, t = divmod(g, 2)
                    pe.wait_ge(xsA[w % 2], 16 * (w // 2 + 1))
                    if t == 1:
                        pe.wait_ge(xsB[w % 2], 16 * (w // 2 + 1))
                    if g >= 8:
                        pe.wait_ge(dvsem, g - 7)  # psum bank g%8 evacuated
                    x5 = x5s[w % NSLOT]
                    ps = pss[g % 8]
                    for j, (dy, dx) in enumerate(TAPS_XY):
                        jy, py = divmod(dy + 2, 2)
                        jx, px = divmod(dx + 2, 2)
                        a0 = 8 * t + jy
                        rhs = x5[
                            :, a0 : a0 + 8, py : py + 1, px : px + 1, jx : jx + 64
                        ]
                        mm = pe.matmul(
                            ps[:],
                            wtile[:, j * 128 : (j + 1) * 128],
                            rhs,
                            start=(j == 0),
                            stop=(j == len(TAPS_XY) - 1),
                        )
                        if j == len(TAPS_XY) - 1:
                            mm.then_inc(pesem, 1)

            @block.vector
            def _(dve):
                for g in range(NG):
                    M = _oz_per(g // 2) * 16
                    if g >= 4:
                        dve.wait_ge(osem, 16 * (g - 3))  # stage slot g%4 free
                    dve.wait_ge(pesem, g + 1)
                    dve.tensor_copy(
                        out=stgs[g % 4][:M, :], in_=pss[g % 8][:M]
                    ).then_inc(dvsem, 1)

    nc.compile()
    return nc


def _get_module():
    global _MODULE
    if _MODULE is None:
        _MODULE = _build_module()
    return _MODULE


def _band_weights(w5):
    """wc[k=(zl*8+ic), j*128 + s*16 + oc] block-banded weights.

    Window-local: output plane s (0..6) of any window reads tile-local planes
    zl = 2*s + dzi (dzi = dz+1, dz in {-1,0,1}); rows 120-127 and M-columns
    112-127 stay zero."""
    wc = np.zeros((128, 9, 8, 16), np.float32)
    for j, (dy, dx) in enumerate(TAPS_XY):
        for dzi in range(3):
            blk = w5[:, :, dzi + 1, dy + 2, dx + 2].T  # [ic, oc]
            for s in range(7):
                zl = 2 * s + dzi
                wc[zl * 8 : (zl + 1) * 8, j, s, :] = blk
    return np.ascontiguousarray(wc.reshape(128, 9 * 128))


def _shard_core_input(x, b, gy):
    """Per-core padded input as 10 z-window tiles [128, 36*132]."""
    xp = np.zeros((IC, 142, 36, 132), BF16)
    y_lo = 32 * gy - 2
    src_lo, src_hi = max(0, y_lo), min(128, y_lo + 36)
    xp[:, 2:130, src_lo - y_lo : src_hi - y_lo, 2:130] = x[
        b, :, :, src_lo:src_hi, :
    ]
    tiles = np.zeros((NW, 128, SUB_FREE), BF16)
    for w in range(NW):
        u = xp[:, 14 * w + 1 : 14 * w + 16]  # [ic, zl 15, y 36, x 132]
        # de-interleave phases: free = (yo 18, yp 2, px 2, xe 66)
        u = u.reshape(IC, 15, 36, 66, 2).transpose(0, 1, 2, 4, 3)
        u = u.reshape(IC, 15, 18, 2, 2, 66)
        tiles[w, :120] = u.transpose(1, 0, 2, 3, 4, 5).reshape(120, SUB_FREE)
    return tiles


def kernel(x, weight, bias, psi_local):
    global LAST_RESULT
    from concourse.bass_utils import run_bass_kernel_spmd

    x = np.asarray(x, np.float32)
    weight = np.asarray(weight, np.float32)
    bias = np.asarray(bias, np.float32)
    psi_local = np.asarray(psi_local, np.float32)

    w5 = np.einsum("ogk,kzyx->ogzyx", weight, psi_local).astype(np.float32)
    wc = _band_weights(w5).astype(BF16)

    in_maps = []
    for core in range(N_CORES):
        b, gy = divmod(core, 4)
        in_maps.append({"xc": _shard_core_input(x, b, gy), "wc": wc})

    nc = _get_module()
    trace = bool(int(os.environ.get("KERNEL_TRACE", "0")))
    res = run_bass_kernel_spmd(
        nc, in_maps, core_ids=list(range(N_CORES)), trace=trace
    )
    LAST_RESULT = res

    out = np.empty((2, OC, 64, 64, 64), np.float32)
    for core in range(N_CORES):
        b, gy = divmod(core, 4)
        out[b, :, :, 16 * gy : 16 * gy + 16] = (
            res.results[core]["out"].astype(np.float32).transpose(1, 0, 2, 3)
        )
    out += bias[None, :, None, None, None]
    return out
